# revision 7
# baseline (speedup 1.0000x reference)
"""BitStackLinear Trainium2 kernel.

Computes out = x @ w.T where w = sum_i sign_i * (u_i @ vt_i), signs unpacked
from 4 packed bit-planes (one byte = 8 signs, little-endian).

Strategy: tensor-parallel over out_features across 8 NeuronCores
(1376 rows each). Per core:

  Phase R (reconstruct w.T into SBUF as bf16, per 128-row k-slab):
    - PE: r_i = vt_i.T @ u_i.T (rank-16 matmuls, 4 bits row-tiled at
      tile_position (32i, 0)) -> PSUM [128k, 1376o] f32
    - sign application: t_i = (a_i - 2^(j-1)) * r_i where a_i = byte & (1<<j),
      j = k%8.  The leftover 2^(j-1) scale is cancelled by scaling x with
      2^(1-j) during its bf16 conversion (per-partition scalar).
      bits 0,1: DVE STT reading r from PSUM directly.
      bits 2,3: ScalarE evacuates r to SBUF bf16, GpSimd STT.
    - w.T slab = sum of the 4 signed terms, kept resident in SBUF (bf16,
      88KB/partition total) - never spilled to DRAM.
  Phase G (GEMM, all-bf16 operands, f32 PSUM accumulation):
    - out.T[o, m] chunk = sum_k w.T[k, o-tile]^T-contraction @ xb[k, m-chunk]
    - xb = x.T converted f32->bf16 on device (with the 2^(1-j) scale),
      m-chunks of 512 double-buffered
    - bf16 stationary enables FWL (fast weight load); matmuls emitted
      back-to-back to hold the PE HAM clock at 2.4 GHz.

kernel(**inputs) takes the full unsharded inputs and returns the full output.
Host work is layout only: transposes, dtype reinterpretation, byte replication
(np.repeat for the broadcast sign bytes), sharding.
"""

import contextlib
import numpy as np

import concourse.bass as bass
import concourse.bacc as bacc
import concourse.mybir as mybir
import concourse.tile as tile

W_BIT = 4
OUT_F = 11008
IN_F = 4096
RANK = 16
NCORES = 8
O_SHARD = OUT_F // NCORES          # 1376
O_TILES = (O_SHARD + 127) // 128   # 11 (last tile 96 wide)
K_TILES = IN_F // 128              # 32
MC = 512                           # m-chunk width


def _bitstack_body(tc, aps, M):
    nc = tc.nc
    xT, qbE, uT, vtp, bm4, hm, ppsx, outT = (
        aps["xT"], aps["qbE"], aps["uT"], aps["vtp"], aps["bm4"], aps["hm"],
        aps["ppsx"], aps["outT"],
    )
    f32, u8, i32 = mybir.dt.float32, mybir.dt.uint8, mybir.dt.int32
    bf16, f32r = mybir.dt.bfloat16, mybir.dt.float32r
    AF = mybir.ActivationFunctionType
    OP = mybir.AluOpType
    n_mb = M // MC
    OS = O_SHARD

    with contextlib.ExitStack() as ctx:
        pool = ctx.enter_context(tc.tile_pool(name="sb", bufs=1))
        psum = ctx.enter_context(tc.tile_pool(name="ps", bufs=1, space="PSUM"))

        # ---- constants ----
        bm4_t = pool.tile([128, W_BIT * OS], u8, name="bm4_t")
        nc.sync.dma_start(bm4_t, bm4)
        hm_t = pool.tile([128, 1], f32, name="hm_t")
        nc.sync.dma_start(hm_t, hm)
        ppsx_t = pool.tile([128, 1], f32, name="ppsx_t")
        nc.sync.dma_start(ppsx_t, ppsx)
        # u.T, 4 bit-planes packed at partitions 32i..32i+16
        utb = pool.tile([128, OS], f32r, name="utb")
        for i in range(W_BIT):
            nc.sync.dma_start(utb[32 * i:32 * i + RANK], uT[i].bitcast(f32r))

        xb_h = {}

        def emit_xload_k(mb, k, use_scalar):
            xs = pool.tile([128, MC], f32, name=f"xs{mb}_{k}", tag="xs", bufs=3)
            nc.sync.dma_start(xs, xT[k * 128:(k + 1) * 128,
                                     mb * MC:(mb + 1) * MC])
            xbt = pool.tile([128, MC], bf16, name=f"xb{mb}_{k}", tag=f"xb{k}",
                            bufs=2)
            if use_scalar:
                nc.scalar.activation(xbt, xs, AF.Copy, scale=ppsx_t)
            else:
                nc.gpsimd.tensor_scalar(out=xbt, in0=xs, scalar1=ppsx_t,
                                        scalar2=None, op0=OP.mult)
            xb_h.setdefault(mb, [None] * K_TILES)[k] = xbt

        # ---- Phase R: reconstruct w.T slabs into SBUF (bf16) ----
        w_tiles = []
        for ks in range(K_TILES):
            vtb = pool.tile([128, 128], f32r, name=f"vtb{ks}", tag="vtb",
                            bufs=4)
            nc.sync.dma_start(vtb, vtp[:, ks * 128:(ks + 1) * 128]
                              .bitcast(f32r))
            bts4 = pool.tile([128, W_BIT * OS], u8, name=f"bts{ks}", tag="bts",
                             bufs=2)
            nc.sync.dma_start(bts4, qbE[ks * 128:(ks + 1) * 128, :])
            prs = []
            for i in range(W_BIT):
                pr = psum.tile([128, OS], f32, name=f"pr{ks}_{i}", tag="pr",
                               bufs=2, padded_shape=[128, 1536])
                for c0 in range(0, OS, 512):
                    c1 = min(c0 + 512, OS)
                    nc.tensor.matmul(
                        pr[:, c0:c1],
                        vtb[32 * i:32 * i + RANK],
                        utb[32 * i:32 * i + RANK, c0:c1],
                        start=True, stop=True,
                        tile_position=(32 * i, 0),
                    )
                prs.append(pr)
            # unpack: a = byte & (1<<j), in-place over i32 lanes (DVE-only op)
            nc.vector.tensor_tensor(
                out=bts4.bitcast(i32), in0=bts4.bitcast(i32),
                in1=bm4_t.bitcast(i32), op=OP.bitwise_and)
            # sign apply t_i = (a_i - 2^(j-1)) * r_i on DVE (STT is DVE-only,
            # r_i read straight from PSUM); plain adds on GpSimd (Pool)
            ts = []
            for i in range(W_BIT):
                t = pool.tile([128, OS], bf16, name=f"t{i}_{ks}", tag=f"t{i}",
                              bufs=2)
                nc.vector.scalar_tensor_tensor(
                    out=t, in0=bts4[:, i * OS:(i + 1) * OS], scalar=hm_t,
                    in1=prs[i], op0=OP.subtract, op1=OP.mult)
                ts.append(t)
            nc.gpsimd.tensor_tensor(out=ts[0], in0=ts[0], in1=ts[1], op=OP.add)
            nc.gpsimd.tensor_tensor(out=ts[2], in0=ts[2], in1=ts[3], op=OP.add)
            wsb = pool.tile([128, OS], bf16, name=f"w{ks}", tag=f"w{ks}",
                            bufs=1)
            nc.gpsimd.tensor_tensor(out=wsb, in0=ts[0], in1=ts[2], op=OP.add)
            w_tiles.append(wsb)
            # interleave x chunk-0/1 loads (ScalarE converts) with recon
            for j in (2 * ks, 2 * ks + 1):
                mb, k = divmod(j, K_TILES)
                if mb < n_mb:
                    emit_xload_k(mb, k, use_scalar=True)

        # ---- Phase G: out.T[o,m] = sum_k w.T[k,o]^T @ xb[k,m] ----
        for mb in range(n_mb):
            pf = mb + 2
            if pf < n_mb:
                for k in range(K_TILES):
                    emit_xload_k(pf, k, use_scalar=False)
            xt = xb_h[mb]
            for ot in range(O_TILES):
                ow = min(128, OS - ot * 128)
                pg = psum.tile([128, MC], f32, name=f"pg{mb}_{ot}", tag="pg",
                               bufs=2)
                for k in range(K_TILES):
                    nc.tensor.matmul(
                        pg[:ow],
                        w_tiles[k][:, ot * 128:ot * 128 + ow],
                        xt[k],
                        start=(k == 0), stop=(k == K_TILES - 1),
                    )
                ob = pool.tile([128, MC], f32, name=f"ob{mb}_{ot}", tag="ob",
                               bufs=2)
                if ot % 2 == 0:
                    nc.scalar.copy(ob[:ow], pg[:ow])
                else:
                    nc.vector.tensor_copy(ob[:ow], pg[:ow])
                nc.sync.dma_start(
                    outT[ot * 128:ot * 128 + ow, mb * MC:(mb + 1) * MC],
                    ob[:ow])
            del xb_h[mb]


def build_bass(M=8192):
    nc = bacc.Bacc("TRN2", target_bir_lowering=False, debug=False)
    f32, u8 = mybir.dt.float32, mybir.dt.uint8
    aps = {}
    aps["xT"] = nc.dram_tensor("xT", [IN_F, M], f32, kind="ExternalInput").ap()
    # sign bytes pre-replicated 8x along k (layout-only np.repeat on host):
    # qbE[k, i*1376 + c] = qweight byte for (bit i, out c, in k)
    aps["qbE"] = nc.dram_tensor("qbE", [IN_F, W_BIT * O_SHARD], u8,
                                kind="ExternalInput").ap()
    aps["uT"] = nc.dram_tensor("uT", [W_BIT, RANK, O_SHARD], f32,
                               kind="ExternalInput").ap()
    # vt bit-planes packed at partitions 32i..32i+16 (zeros elsewhere)
    aps["vtp"] = nc.dram_tensor("vtp", [128, IN_F], f32,
                                kind="ExternalInput").ap()
    aps["bm4"] = nc.dram_tensor("bm4", [128, W_BIT * O_SHARD], u8,
                                kind="ExternalInput").ap()
    aps["hm"] = nc.dram_tensor("hm", [128, 1], f32, kind="ExternalInput").ap()
    aps["ppsx"] = nc.dram_tensor("ppsx", [128, 1], f32,
                                 kind="ExternalInput").ap()
    aps["outT"] = nc.dram_tensor("outT", [O_SHARD, M], f32,
                                 kind="ExternalOutput").ap()
    with tile.TileContext(nc) as tc:
        _bitstack_body(tc, aps, M)
    nc.compile()
    return nc


def prep_inputs(x, qweight, u, vt):
    """Host-side layout prep (transposes / dtype views / replication only)."""
    M = x.shape[0] * x.shape[1]
    xT = np.ascontiguousarray(x.reshape(M, IN_F).T)
    qb = qweight.astype(np.uint8)  # values 0..255 stored in int32
    p = np.arange(128)
    bm = (np.uint8(1) << (p % 8).astype(np.uint8))[:, None] * np.ones(
        (1, W_BIT * O_SHARD), np.uint8)
    hm = (2.0 ** ((p % 8) - 1.0)).astype(np.float32).reshape(128, 1)
    ppsx = (2.0 ** (1.0 - (p % 8))).astype(np.float32).reshape(128, 1)
    # vt packed: partition 32i+r holds vt[i, r, :]
    vtp = np.zeros((128, IN_F), np.float32)
    for i in range(W_BIT):
        vtp[32 * i:32 * i + RANK] = vt[i]
    in_maps = []
    qb_r = qb.reshape(W_BIT, OUT_F, IN_F // 8)
    for c in range(NCORES):
        sl = slice(c * O_SHARD, (c + 1) * O_SHARD)
        # [bit, 512 bytes, o] -> replicate each byte row 8x -> [4096, o]
        qbT = qb_r[:, sl, :].transpose(0, 2, 1)          # [4, 512, 1376]
        qbE = np.ascontiguousarray(
            np.repeat(qbT, 8, axis=1).transpose(1, 0, 2).reshape(
                IN_F, W_BIT * O_SHARD))
        uT = np.ascontiguousarray(u[:, sl, :].transpose(0, 2, 1))
        in_maps.append({
            "xT": xT, "qbE": qbE, "uT": uT, "vtp": vtp,
            "bm4": bm, "hm": hm, "ppsx": ppsx,
        })
    return in_maps


def _enable_ldw_opt():
    """No-op (kept for test.py compat). Walrus ldw-opt rejects the
    tile_position LDWEIGHTS used by the row-tiled recon matmuls, and the
    GEMM has no consecutive same-stationary matmuls to dedup anyway."""


def kernel(x, qweight, u, vt):
    from concourse import bass_utils
    _enable_ldw_opt()
    x = np.asarray(x)
    qweight = np.asarray(qweight)
    u = np.asarray(u)
    vt = np.asarray(vt)
    B, S, _ = x.shape
    M = B * S
    nc = build_bass(M)
    in_maps = prep_inputs(x, qweight, u, vt)
    res = bass_utils.run_bass_kernel_spmd(nc, in_maps, core_ids=list(range(NCORES)))
    out = np.empty((M, OUT_F), np.float32)
    for c in range(NCORES):
        out[:, c * O_SHARD:(c + 1) * O_SHARD] = res.results[c]["outT"].T
    return out.reshape(B, S, OUT_F)


if __name__ == "__main__":
    rng = np.random.default_rng(0)
    x = rng.standard_normal((4, 2048, IN_F)).astype(np.float32)
    qw = rng.integers(0, 256, size=(W_BIT, OUT_F * IN_F // 8)).astype(np.int32)
    uu = (rng.standard_normal((W_BIT, OUT_F, RANK)) * 0.05).astype(np.float32)
    vv = (rng.standard_normal((W_BIT, RANK, IN_F)) * 0.05).astype(np.float32)
    out = kernel(x=x, qweight=qw, u=uu, vt=vv)
    print(out.shape, out.dtype)


# revision 18
# speedup vs baseline: 2.6167x; 2.6167x over previous
"""BitStackLinear Trainium2 kernel.

Computes out = x @ w.T where w = sum_i sign_i * (u_i @ vt_i), signs unpacked
from 4 packed bit-planes (one byte = 8 signs, little-endian).

Strategy: tensor-parallel over out_features across 8 NeuronCores
(1376 rows each). Per core:

  Phase R (reconstruct w.T into SBUF as bf16, per 128-row k-slab):
    - PE: r_i = vt_i.T @ u_i.T (rank-16 matmuls, 4 bits row-tiled at
      tile_position (32i, 0)) -> PSUM [128k, 1376o] f32
    - sign application: t_i = (a_i - 2^(j-1)) * r_i where a_i = byte & (1<<j),
      j = k%8.  The leftover 2^(j-1) scale is cancelled by scaling x with
      2^(1-j) during its bf16 conversion (per-partition scalar).
      bits 0,1: DVE STT reading r from PSUM directly.
      bits 2,3: ScalarE evacuates r to SBUF bf16, GpSimd STT.
    - w.T slab = sum of the 4 signed terms, kept resident in SBUF (bf16,
      88KB/partition total) - never spilled to DRAM.
  Phase G (GEMM, all-bf16 operands, f32 PSUM accumulation):
    - out.T[o, m] chunk = sum_k w.T[k, o-tile]^T-contraction @ xb[k, m-chunk]
    - xb = x.T converted f32->bf16 on device (with the 2^(1-j) scale),
      m-chunks of 512 double-buffered
    - bf16 stationary enables FWL (fast weight load); matmuls emitted
      back-to-back to hold the PE HAM clock at 2.4 GHz.

kernel(**inputs) takes the full unsharded inputs and returns the full output.
Host work is layout only: transposes, dtype reinterpretation, byte replication
(np.repeat for the broadcast sign bytes), sharding.
"""

import contextlib
import numpy as np

import concourse.bass as bass
import concourse.bacc as bacc
import concourse.mybir as mybir
import concourse.tile as tile

W_BIT = 4
OUT_F = 11008
IN_F = 4096
RANK = 16
NCORES = 8
O_SHARD = OUT_F // NCORES          # 1376
O_TILES = (O_SHARD + 127) // 128   # 11 (last tile 96 wide)
K_TILES = IN_F // 128              # 32
MC = 512                           # m-chunk width


def _bitstack_body(tc, aps, M):
    nc = tc.nc
    xT, qbE, uT, vtp, bm4, hm, ppsx, outT = (
        aps["xT"], aps["qbE"], aps["uT"], aps["vtp"], aps["bm4"], aps["hm"],
        aps["ppsx"], aps["outT"],
    )
    f32, u8, i32 = mybir.dt.float32, mybir.dt.uint8, mybir.dt.int32
    bf16, f32r = mybir.dt.bfloat16, mybir.dt.float32r
    AF = mybir.ActivationFunctionType
    OP = mybir.AluOpType
    n_mb = M // MC
    OS = O_SHARD

    with contextlib.ExitStack() as ctx:
        pool = ctx.enter_context(tc.tile_pool(name="sb", bufs=1))
        psum = ctx.enter_context(tc.tile_pool(name="ps", bufs=1, space="PSUM"))

        # ---- constants ----
        bm4_t = pool.tile([128, W_BIT * OS], u8, name="bm4_t")
        nc.sync.dma_start(bm4_t, bm4)
        hm_t = pool.tile([128, 1], f32, name="hm_t")
        nc.sync.dma_start(hm_t, hm)
        ppsx_t = pool.tile([128, 1], f32, name="ppsx_t")
        nc.sync.dma_start(ppsx_t, ppsx)
        # u.T, 4 bit-planes packed at partitions 32i..32i+16
        utb = pool.tile([128, OS], f32r, name="utb")
        for i in range(W_BIT):
            nc.sync.dma_start(utb[32 * i:32 * i + RANK], uT[i].bitcast(f32r))

        xb_h = {}

        def emit_xload_k(mb, k, use_scalar):
            xs = pool.tile([128, MC], f32, name=f"xs{mb}_{k}", tag="xs", bufs=3)
            nc.sync.dma_start(xs, xT[k * 128:(k + 1) * 128,
                                     mb * MC:(mb + 1) * MC])
            xbt = pool.tile([128, MC], bf16, name=f"xb{mb}_{k}", tag=f"xb{k}",
                            bufs=2)
            if use_scalar:
                nc.scalar.activation(xbt, xs, AF.Copy, scale=ppsx_t)
            else:
                nc.vector.tensor_scalar(out=xbt, in0=xs, scalar1=ppsx_t,
                                        scalar2=None, op0=OP.mult)
            xb_h.setdefault(mb, [None] * K_TILES)[k] = xbt

        # ---- Phase R: reconstruct w.T slabs into SBUF (bf16) ----
        w_tiles = []
        for ks in range(K_TILES):
            vtb = pool.tile([128, 128], f32r, name=f"vtb{ks}", tag="vtb",
                            bufs=4)
            nc.sync.dma_start(vtb, vtp[:, ks * 128:(ks + 1) * 128]
                              .bitcast(f32r))
            bts4 = pool.tile([128, W_BIT * OS], u8, name=f"bts{ks}", tag="bts",
                             bufs=2)
            nc.sync.dma_start(bts4, qbE[ks * 128:(ks + 1) * 128, :])
            prs = []
            for i in range(W_BIT):
                pr = psum.tile([128, OS], f32, name=f"pr{ks}_{i}", tag="pr",
                               bufs=2, padded_shape=[128, 1536])
                for c0 in range(0, OS, 512):
                    c1 = min(c0 + 512, OS)
                    nc.tensor.matmul(
                        pr[:, c0:c1],
                        vtb[32 * i:32 * i + RANK],
                        utb[32 * i:32 * i + RANK, c0:c1],
                        start=True, stop=True,
                        tile_position=(32 * i, 0),
                    )
                prs.append(pr)
            # unpack: a = byte & (1<<j), in-place over i32 lanes (DVE-only op)
            nc.vector.tensor_tensor(
                out=bts4.bitcast(i32), in0=bts4.bitcast(i32),
                in1=bm4_t.bitcast(i32), op=OP.bitwise_and)
            # sign apply t_i = (a_i - 2^(j-1)) * r_i on DVE (STT is DVE-only,
            # r_i read straight from PSUM); plain adds on GpSimd (Pool)
            ts = []
            for i in range(W_BIT):
                t = pool.tile([128, OS], bf16, name=f"t{i}_{ks}", tag=f"t{i}",
                              bufs=2)
                nc.vector.scalar_tensor_tensor(
                    out=t, in0=bts4[:, i * OS:(i + 1) * OS], scalar=hm_t,
                    in1=prs[i], op0=OP.subtract, op1=OP.mult)
                ts.append(t)
            nc.gpsimd.tensor_tensor(out=ts[0], in0=ts[0], in1=ts[1], op=OP.add)
            nc.gpsimd.tensor_tensor(out=ts[2], in0=ts[2], in1=ts[3], op=OP.add)
            wsb = pool.tile([128, OS], bf16, name=f"w{ks}", tag=f"w{ks}",
                            bufs=1)
            nc.vector.tensor_tensor(out=wsb, in0=ts[0], in1=ts[2], op=OP.add)
            w_tiles.append(wsb)
            # interleave x chunk-0/1 loads (ScalarE converts) with recon
            for j in (2 * ks, 2 * ks + 1):
                mb, k = divmod(j, K_TILES)
                if mb < n_mb:
                    emit_xload_k(mb, k, use_scalar=True)

        # ---- Phase G: out.T[o,m] = sum_k w.T[k,o]^T @ xb[k,m] ----
        for mb in range(n_mb):
            pf = mb + 2
            if pf < n_mb:
                for k in range(K_TILES):
                    emit_xload_k(pf, k, use_scalar=(k % 2 == 0))
            xt = xb_h[mb]
            for ot in range(O_TILES):
                ow = min(128, OS - ot * 128)
                pg = psum.tile([128, MC], f32, name=f"pg{mb}_{ot}", tag="pg",
                               bufs=2)
                for k in range(K_TILES):
                    nc.tensor.matmul(
                        pg[:ow],
                        w_tiles[k][:, ot * 128:ot * 128 + ow],
                        xt[k],
                        start=(k == 0), stop=(k == K_TILES - 1),
                    )
                ob = pool.tile([128, MC], f32, name=f"ob{mb}_{ot}", tag="ob",
                               bufs=2)
                if ot % 2 == 0:
                    nc.scalar.copy(ob[:ow], pg[:ow])
                else:
                    nc.vector.tensor_copy(ob[:ow], pg[:ow])
                nc.sync.dma_start(
                    outT[ot * 128:ot * 128 + ow, mb * MC:(mb + 1) * MC],
                    ob[:ow])
            del xb_h[mb]


def build_bass(M=8192):
    nc = bacc.Bacc("TRN2", target_bir_lowering=False, debug=False)
    f32, u8 = mybir.dt.float32, mybir.dt.uint8
    aps = {}
    aps["xT"] = nc.dram_tensor("xT", [IN_F, M], f32, kind="ExternalInput").ap()
    # sign bytes pre-replicated 8x along k (layout-only np.repeat on host):
    # qbE[k, i*1376 + c] = qweight byte for (bit i, out c, in k)
    aps["qbE"] = nc.dram_tensor("qbE", [IN_F, W_BIT * O_SHARD], u8,
                                kind="ExternalInput").ap()
    aps["uT"] = nc.dram_tensor("uT", [W_BIT, RANK, O_SHARD], f32,
                               kind="ExternalInput").ap()
    # vt bit-planes packed at partitions 32i..32i+16 (zeros elsewhere)
    aps["vtp"] = nc.dram_tensor("vtp", [128, IN_F], f32,
                                kind="ExternalInput").ap()
    aps["bm4"] = nc.dram_tensor("bm4", [128, W_BIT * O_SHARD], u8,
                                kind="ExternalInput").ap()
    aps["hm"] = nc.dram_tensor("hm", [128, 1], f32, kind="ExternalInput").ap()
    aps["ppsx"] = nc.dram_tensor("ppsx", [128, 1], f32,
                                 kind="ExternalInput").ap()
    aps["outT"] = nc.dram_tensor("outT", [O_SHARD, M], f32,
                                 kind="ExternalOutput").ap()
    with tile.TileContext(nc) as tc:
        _bitstack_body(tc, aps, M)
    nc.compile()
    return nc


def prep_inputs(x, qweight, u, vt):
    """Host-side layout prep (transposes / dtype views / replication only)."""
    M = x.shape[0] * x.shape[1]
    xT = np.ascontiguousarray(x.reshape(M, IN_F).T)
    qb = qweight.astype(np.uint8)  # values 0..255 stored in int32
    p = np.arange(128)
    bm = (np.uint8(1) << (p % 8).astype(np.uint8))[:, None] * np.ones(
        (1, W_BIT * O_SHARD), np.uint8)
    hm = (2.0 ** ((p % 8) - 1.0)).astype(np.float32).reshape(128, 1)
    ppsx = (2.0 ** (1.0 - (p % 8))).astype(np.float32).reshape(128, 1)
    # vt packed: partition 32i+r holds vt[i, r, :]
    vtp = np.zeros((128, IN_F), np.float32)
    for i in range(W_BIT):
        vtp[32 * i:32 * i + RANK] = vt[i]
    in_maps = []
    qb_r = qb.reshape(W_BIT, OUT_F, IN_F // 8)
    for c in range(NCORES):
        sl = slice(c * O_SHARD, (c + 1) * O_SHARD)
        # [bit, 512 bytes, o] -> replicate each byte row 8x -> [4096, o]
        qbT = qb_r[:, sl, :].transpose(0, 2, 1)          # [4, 512, 1376]
        qbE = np.ascontiguousarray(
            np.repeat(qbT, 8, axis=1).transpose(1, 0, 2).reshape(
                IN_F, W_BIT * O_SHARD))
        uT = np.ascontiguousarray(u[:, sl, :].transpose(0, 2, 1))
        in_maps.append({
            "xT": xT, "qbE": qbE, "uT": uT, "vtp": vtp,
            "bm4": bm, "hm": hm, "ppsx": ppsx,
        })
    return in_maps


def _enable_ldw_opt():
    """No-op (kept for test.py compat). Walrus ldw-opt rejects the
    tile_position LDWEIGHTS used by the row-tiled recon matmuls, and the
    GEMM has no consecutive same-stationary matmuls to dedup anyway."""


def kernel(x, qweight, u, vt):
    from concourse import bass_utils
    _enable_ldw_opt()
    x = np.asarray(x)
    qweight = np.asarray(qweight)
    u = np.asarray(u)
    vt = np.asarray(vt)
    B, S, _ = x.shape
    M = B * S
    nc = build_bass(M)
    in_maps = prep_inputs(x, qweight, u, vt)
    res = bass_utils.run_bass_kernel_spmd(nc, in_maps, core_ids=list(range(NCORES)))
    out = np.empty((M, OUT_F), np.float32)
    for c in range(NCORES):
        out[:, c * O_SHARD:(c + 1) * O_SHARD] = res.results[c]["outT"].T
    return out.reshape(B, S, OUT_F)


if __name__ == "__main__":
    rng = np.random.default_rng(0)
    x = rng.standard_normal((4, 2048, IN_F)).astype(np.float32)
    qw = rng.integers(0, 256, size=(W_BIT, OUT_F * IN_F // 8)).astype(np.int32)
    uu = (rng.standard_normal((W_BIT, OUT_F, RANK)) * 0.05).astype(np.float32)
    vv = (rng.standard_normal((W_BIT, RANK, IN_F)) * 0.05).astype(np.float32)
    out = kernel(x=x, qweight=qw, u=uu, vt=vv)
    print(out.shape, out.dtype)
